# revision 14
# baseline (speedup 1.0000x reference)
"""Trainium2 Bass kernel for a single-head causal attention block.

Computes, per batch b:
    xn    = LayerNorm(x[b])           (non-affine, eps=1e-6)
    q,k,v = xn @ Wq, xn @ Wk, xn @ Wv
    s     = causal_mask(q @ k.T / sqrt(D))
    out   = softmax(s) @ v @ Wo + x[b]

Sharding (8 cores, SPMD single program):
  core c -> batch b = c//4, query stripe j = c%4 (queries {4k+j}).
  K/V ownership is chunked so the AllGather pipelines in two halves:
  core j computes K^T and V for tokens [512j, 512(j+1)) (chunk 1) and
  [2048+512j, 2048+512(j+1)) (chunk 2).  AllGather #1 delivers keys
  [0,2048) while chunk-2 K/V is still being computed; AllGather #2
  delivers keys [2048,4096) while attention on the first half runs.
  K is gathered already transposed (computed as K^T = Wk-tile.T @ xnT,
  same orientation as Q^T) so the attention phase needs no transposes.

Attention uses TQ=256 query groups (group g covers original positions
[1024g, 1024(g+1)) via the stride-4 query assignment): groups 0/1
depend only on AllGather #1; groups 2/3 accumulate P@V partial sums
over the first 16 key tiles inside the AllGather #2 window (saved to
bf16) and combine with the upper half afterwards.  Score tiles in the
causal-diagonal region with >=50% masked queries run at half moving
width.  Softmax denominators come from an all-ones stationary matmul
over pT accumulated in PSUM (lagged one tile behind exp so the PE
never waits on the Act engine); exp() is applied without max
subtraction (scores are O(10); fp32 exp cannot overflow), matching
softmax exactly after normalization.

The causal mask is shift-invariant across diagonal tiles:
mask[u][r, q] = 0 iff r <= 4q + j - 128u, so a single [128, 480] tile
m[r, s] = 0 iff r <= 4s + j - 896 serves every tile u via the column
slice [32*(7-u), 32*(7-u)+256).
"""

import numpy as np
import ml_dtypes

import concourse.bacc as bacc
import concourse.tile as tile
from concourse import mybir
from concourse.bass_utils import run_bass_kernel_spmd

# Problem shape (hardcoded per harness contract)
B, S, H, D = 2, 4096, 2048, 2048
NCORES = 8
P = 128            # partitions
GQ = NCORES // B   # cores per batch = query stride
SQ = S // GQ       # queries per core
TQ = 256           # query group width
NGRP = SQ // TQ    # query groups per core (4)
HT = H // P        # h tiles (16)
DT = D // P        # d tiles (16)
NDIAG = TQ * GQ // P   # diagonal (mask) key tiles per query group (8)
MW = TQ + 32 * (NDIAG - 1)  # shared mask tile width (480)
CH = 512           # tokens per core per chunk
CHEL = CH * D      # elements per core-chunk contribution (per tensor)

F32 = mybir.dt.float32
BF16 = mybir.dt.bfloat16
CDT = BF16
CDT_NP = ml_dtypes.bfloat16

EPS = 1e-6
NEG = -1e30


def build_nc(compile=True):
    nc = bacc.Bacc(num_devices=NCORES)

    # I/O.  xkv rows = [chunk1 tokens ; chunk2 tokens] for this core.
    xkv = nc.dram_tensor("xkv", [2 * CH, H], CDT, kind="ExternalInput")
    xq = nc.dram_tensor("xq", [SQ, H], CDT, kind="ExternalInput")
    wq = nc.dram_tensor("wq", [H, D], CDT, kind="ExternalInput")
    wk = nc.dram_tensor("wk", [H, D], CDT, kind="ExternalInput")
    wv = nc.dram_tensor("wv", [H, D], CDT, kind="ExternalInput")
    wo = nc.dram_tensor("wo", [D, H], CDT, kind="ExternalInput")
    masks = nc.dram_tensor("masks", [P, MW], F32, kind="ExternalInput")
    ident = nc.dram_tensor("ident", [P, P], CDT, kind="ExternalInput")
    out = nc.dram_tensor("out", [SQ, H], F32, kind="ExternalOutput")

    # DRAM scratch
    xn_d = nc.dram_tensor("xn_d", [2 * CH, H], CDT)
    xqn_d = nc.dram_tensor("xqn_d", [SQ, H], CDT)
    qt_d = nc.dram_tensor("qt_d", [D, SQ], CDT)
    # Per-chunk gather buffers: [0] = K^T as [DT,128,CH], [1] = V as [CH,D]
    agin = [nc.dram_tensor(f"agin{c}", [2, CHEL], CDT) for c in range(2)]
    agout = [nc.dram_tensor(f"agout{c}", [GQ, 2, CHEL], CDT) for c in range(2)]
    CC_GROUPS = [list(range(g * GQ, (g + 1) * GQ)) for g in range(NCORES // GQ)]

    RSCALE = float(1.0 / np.sqrt(D))

    with (
        tile.TileContext(nc, pool_alloc_mode="queue") as tc,
        tc.tile_pool(name="consts", bufs=1) as consts,
        tc.tile_pool(name="wvo", bufs=1) as wvo_p,    # Wv then Wo slots
    ):
        ones = consts.tile([P, P], CDT)
        nc.vector.memset(ones, 1.0)
        eps_tile = consts.tile([P, 1], F32)
        nc.vector.memset(eps_tile, EPS)
        masks_sb = consts.tile([P, MW], F32)
        ident_sb = consts.tile([P, P], CDT)

        def load_w(pool, w_dram, prefix, queues):
            tiles = []
            for a in range(HT):
                t = pool.tile([P, w_dram.shape[1]], CDT, tag=f"{prefix}{a}")
                q = queues[a % len(queues)]
                q.dma_start(out=t, in_=w_dram[a * P : (a + 1) * P, :])
                tiles.append(t)
            return tiles

        def load_xt(pool, src_d, row0):
            """[H, 512] block of x_norm^T via DMA crossbar transpose."""
            tiles = []
            for a in range(HT):
                t = pool.tile([P, CH], CDT, tag=f"xt{a}")
                nc.scalar.dma_start_transpose(
                    t, src_d[row0 : row0 + CH, a * P : (a + 1) * P]
                )
                tiles.append(t)
            return tiles

        # ======== Phase 1: LayerNorm, projections, gathers ========
        with (
            tc.tile_pool(name="wkq", bufs=1) as wkq_p,   # Wk then Wq slots
            tc.tile_pool(name="xnT", bufs=3) as xnT_p,
            tc.tile_pool(name="pp1", bufs=5, space="PSUM") as pp1,
            tc.tile_pool(name="ppt", bufs=2, space="PSUM") as ppt,
            tc.tile_pool(name="xtmp", bufs=2) as xtmp_p,
        ):
            with (
                tc.tile_pool(name="xpool", bufs=2) as xpool,
                tc.tile_pool(name="xnpool", bufs=2) as xnpool,
                tc.tile_pool(name="stats", bufs=4) as stats_p,
                tc.tile_pool(name="small", bufs=8) as small_p,
                tc.tile_pool(name="stage1", bufs=6) as stage_p,
            ):
                def ln_rows(src, dst, t0, nt, q=None):
                    """LayerNorm token tiles [t0, t0+nt) of src -> dst.
                    All x loads issued before any store (HOL-blocking)."""
                    q = q or nc.sync
                    xts = []
                    for t in range(t0, t0 + nt):
                        x_t = xpool.tile([P, H], CDT, tag="x")
                        q.dma_start(
                            out=x_t, in_=src[t * P : (t + 1) * P, :]
                        )
                        xts.append(x_t)
                    for i, t in enumerate(range(t0, t0 + nt)):
                        x_t = xts[i]
                        stats = stats_p.tile([P, H // 512, 6], F32, tag="st")
                        for k in range(H // 512):
                            nc.vector.bn_stats(
                                out=stats[:, k, :],
                                in_=x_t[:, k * 512 : (k + 1) * 512],
                            )
                        mv = small_p.tile([P, 2], F32, tag="mv")
                        nc.vector.bn_aggr(out=mv, in_=stats)
                        sq = small_p.tile([P, 1], F32, tag="sq")
                        nc.scalar.activation(
                            out=sq, in_=mv[:, 1:2],
                            func=mybir.ActivationFunctionType.Sqrt,
                            bias=eps_tile, scale=1.0,
                        )
                        rs = small_p.tile([P, 1], F32, tag="rs")
                        nc.vector.reciprocal(out=rs, in_=sq)
                        xn_t = xnpool.tile([P, H], CDT, tag="xn")
                        nc.vector.tensor_scalar(
                            out=xn_t, in0=x_t, scalar1=mv[:, 0:1], scalar2=rs,
                            op0=mybir.AluOpType.subtract,
                            op1=mybir.AluOpType.mult,
                        )
                        q.dma_start(
                            out=dst[t * P : (t + 1) * P, :], in_=xn_t
                        )

                def proj_kv(xt, ch):
                    """K^T and V projections for one chunk into agin[ch]."""
                    ktv = agin[ch][0, :].rearrange(
                        "(a p k) -> a p k", p=P, k=CH
                    )
                    vv = agin[ch][1, :].rearrange("(t d) -> t d", d=D)
                    # K^T: [128d, 512tok] tiles
                    for a in range(DT):
                        ps = pp1.tile([P, CH], F32, tag="ps")
                        for h in range(HT):
                            nc.tensor.matmul(
                                ps, wk_sb[h][:, a * P : (a + 1) * P], xt[h],
                                start=(h == 0), stop=(h == HT - 1),
                            )
                        st = stage_p.tile([P, CH], CDT, tag="st")
                        nc.vector.tensor_copy(st, ps)
                        nc.sync.dma_start(out=ktv[a, :, :], in_=st)
                    # V: [128tok, 512d] tiles
                    for tl in range(CH // P):
                        for dc in range(D // 512):
                            ps = pp1.tile([P, 512], F32, tag="ps")
                            for h in range(HT):
                                nc.tensor.matmul(
                                    ps,
                                    xt[h][:, tl * P : (tl + 1) * P],
                                    wv_sb[h][:, dc * 512 : (dc + 1) * 512],
                                    start=(h == 0), stop=(h == HT - 1),
                                )
                            st = stage_p.tile([P, 512], CDT, tag="st")
                            nc.vector.tensor_copy(st, ps)
                            nc.sync.dma_start(
                                out=vv[tl * P : (tl + 1) * P,
                                       dc * 512 : (dc + 1) * 512],
                                in_=st,
                            )

                def proj_q(xt, qb):
                    """Q^T for query block qb (scaled by 1/sqrt(D))."""
                    for a in range(DT):
                        ps = pp1.tile([P, CH], F32, tag="ps")
                        for h in range(HT):
                            nc.tensor.matmul(
                                ps, wq_sb[h][:, a * P : (a + 1) * P], xt[h],
                                start=(h == 0), stop=(h == HT - 1),
                            )
                        st = stage_p.tile([P, CH], CDT, tag="st")
                        nc.scalar.activation(
                            out=st, in_=ps,
                            func=mybir.ActivationFunctionType.Copy,
                            bias=0.0, scale=RSCALE,
                        )
                        nc.sync.dma_start(
                            out=qt_d[a * P : (a + 1) * P,
                                     qb * CH : (qb + 1) * CH],
                            in_=st,
                        )

                def gather(ch):
                    nc.gpsimd.collective_compute(
                        "AllGather", mybir.AluOpType.bypass,
                        replica_groups=CC_GROUPS,
                        ins=[agin[ch][:, :]], outs=[agout[ch][:, :, :]],
                    )

                def tr_q(qb):
                    """xqn^T for query block qb via PE transposes (DMA
                    crossbar transposes serialize with collectives in the
                    scheduler's virtual queue -- any transpose whose pass
                    time races an AllGather stalls the DMA rings at
                    runtime, so only the early chunk transposes use the
                    crossbar)."""
                    tiles = [
                        xnT_p.tile([P, CH], CDT, tag=f"xt{a}",
                                   name=f"xtq{qb}_{a}")
                        for a in range(HT)
                    ]
                    for t in range(CH // P):
                        xm = xtmp_p.tile([P, H], CDT, tag="xm")
                        nc.gpsimd.dma_start(
                            out=xm,
                            in_=xqn_d[qb * CH + t * P : qb * CH + (t + 1) * P, :],
                        )
                        for a in range(HT):
                            ps = ppt.tile([P, P], CDT, tag="pt")
                            nc.tensor.transpose(
                                ps, xm[:, a * P : (a + 1) * P], ident_sb
                            )
                            nc.vector.tensor_copy(
                                tiles[a][:, t * P : (t + 1) * P], ps
                            )
                    return tiles

                wk_sb = load_w(wkq_p, wk, "k", [nc.gpsimd])
                nc.sync.dma_start(out=ident_sb, in_=ident[:, :])
                ln_rows(xkv, xn_d, 0, 4)
                ln_rows(xkv, xn_d, 4, 4)
                ln_rows(xq, xqn_d, 0, 4, q=nc.scalar)
                ln_rows(xq, xqn_d, 4, 4, q=nc.scalar)
                nc.sync.dma_start(out=masks_sb, in_=masks[:, :])
                wv_sb = load_w(wvo_p, wv, "v", [nc.gpsimd])

                xt_c1 = load_xt(xnT_p, xn_d, 0)
                xt_c2 = load_xt(xnT_p, xn_d, CH)
                proj_kv(xt_c1, 0)
                gather(0)
                # Wq on the (idle) gpsimd queue: its slot-reuse wait on the
                # last Wk reader would head-of-line-block SP or Act.
                wq_sb = load_w(wkq_p, wq, "k", [nc.gpsimd])
                proj_kv(xt_c2, 1)
                xt_q1 = tr_q(0)
                proj_q(xt_q1, 0)
                xt_q2 = tr_q(1)
                proj_q(xt_q2, 1)
                gather(1)

        wo_sb = load_w(wvo_p, wo, "v", [nc.gpsimd])  # reuse Wv slots

        # ======== Phase 2: attention ========
        def kt_batch(ktc_p, kc):
            """The 16 kT d-tiles for key batch kc (keys [512kc,+512))."""
            c, r = divmod(kc, 4)
            kv = agout[c][r, 0, :].rearrange("(a p k) -> a p k", p=P, k=CH)
            q = nc.sync if kc % 2 == 0 else nc.scalar
            kts = []
            for a in range(DT):
                t = ktc_p.tile([P, CH], CDT, tag=f"kt{a}")
                q.dma_start(out=t, in_=kv[a, :, :])
                kts.append(t)
            return kts

        def vt_load(vst_p, tk, d0):
            """V tile [128tok, 512] for key tile tk, d cols [d0,d0+512)."""
            c, q = divmod(tk, 16)
            vv = agout[c][q // 4, 1, :].rearrange("(t d) -> t d", d=D)
            row0 = (q % 4) * P
            t = vst_p.tile([P, 512], CDT, tag="vt")
            nc.scalar.dma_start(out=t, in_=vv[row0 : row0 + P, d0 : d0 + 512])
            return t

        def m2_part(ktc_p, psc, g, qg, pT, sums, tk0, tk1):
            """Score pass for group g over key tiles [tk0, tk1)."""
            TK = NDIAG * (g + 1)
            gh = g % 2
            lag = []

            def flush_lag():
                for s_tk, s_qoff, s_nw in lag:
                    nc.tensor.matmul(
                        sums[:, s_qoff : s_qoff + s_nw], ones,
                        pT[:, s_tk, gh, s_qoff : s_qoff + s_nw],
                        start=(s_tk == 0), stop=(s_tk == TK - 1),
                        skip_group_check=True,
                    )
                lag.clear()

            for kc in range(tk0 // 4, tk1 // 4):
                kts = kt_batch(ktc_p, kc)
                for t4 in range(4):
                    tk = kc * 4 + t4
                    u = tk - (TK - NDIAG)
                    qoff = P if (u >= 4) else 0
                    ps = psc.tile([P, TQ], F32, tag="ps")
                    for a in range(DT):
                        nc.tensor.matmul(
                            ps[:, qoff:],
                            kts[a][:, t4 * P : (t4 + 1) * P],
                            qg[:, a, qoff:],
                            start=(a == 0), stop=(a == DT - 1),
                        )
                    if u >= 0:
                        s0 = 32 * (NDIAG - 1 - u)
                        nc.vector.tensor_add(
                            out=ps[:, qoff:], in0=ps[:, qoff:],
                            in1=masks_sb[:, s0 + qoff : s0 + TQ],
                        )
                        if qoff:
                            nc.vector.memset(pT[:, tk, gh, 0:qoff], 0.0)
                    nc.scalar.activation(
                        out=pT[:, tk, gh, qoff:], in_=ps[:, qoff:],
                        func=mybir.ActivationFunctionType.Exp,
                    )
                    flush_lag()
                    lag.append((tk, qoff, TQ - qoff))
            flush_lag()

        def m3_sessions(vst_p, poa_p, pr, pT, tk0, tk1, sink):
            """P@V quarter-sessions for group pair pr over key tiles
            [tk0,tk1).  Tiles >= t_lo only feed the odd group.
            sink(a, psum_tile) consumes each finished d-tile."""
            t_lo = NDIAG * (2 * pr + 1)
            t_dg = NDIAG * (2 * pr + 1)  # odd group's diag base
            for qd in range(4):
                poas = [
                    poa_p.tile([P, 2, TQ], F32, tag=f"poa{d4}",
                               name=f"poa{pr}_{qd}_{d4}_{tk0}")
                    for d4 in range(4)
                ]
                for tk in range(tk0, tk1):
                    vt = vt_load(vst_p, tk, qd * 512)
                    for d4 in range(4):
                        if tk < t_lo:
                            o, r = poas[d4], pT[:, tk, :, :]
                        else:
                            qoff = P if (tk - t_dg >= 4) else 0
                            o = poas[d4][:, 1, qoff:]
                            r = pT[:, tk, 1, qoff:]
                        nc.tensor.matmul(
                            o, vt[:, d4 * P : (d4 + 1) * P], r,
                            start=(tk == tk0), stop=(tk == tk1 - 1),
                            skip_group_check=True,
                        )
                for d4 in range(4):
                    sink(qd * 4 + d4, poas[d4])

        def m4_group(res_p, ost_p, pfin, g, oaT):
            gh = g % 2
            for t2 in range(TQ // P):
                row0 = g * TQ + t2 * P
                for hc in range(H // 512):
                    ps = pfin.tile([P, 512], F32, tag="ps")
                    for d in range(DT):
                        nc.tensor.matmul(
                            ps,
                            oaT[:, d, gh, t2 * P : (t2 + 1) * P],
                            wo_sb[d][:, hc * 512 : (hc + 1) * 512],
                            start=(d == 0), stop=(d == DT - 1),
                        )
                    res = res_p.tile([P, 512], CDT, tag="res")
                    nc.sync.dma_start(
                        out=res,
                        in_=xq[row0 : row0 + P, hc * 512 : (hc + 1) * 512],
                    )
                    ot = ost_p.tile([P, 512], F32, tag="ot")
                    nc.vector.tensor_add(out=ot, in0=ps, in1=res)
                    nc.sync.dma_start(
                        out=out[row0 : row0 + P, hc * 512 : (hc + 1) * 512],
                        in_=ot,
                    )

        def load_qg(qg_p, g):
            t = qg_p.tile([P, DT, TQ], CDT, tag="qg", name=f"qg{g}")
            nc.sync.dma_start(
                out=t,
                in_=qt_d[:, g * TQ : (g + 1) * TQ].rearrange(
                    "(a p) t -> p a t", p=P
                ),
            )
            return t

        with (
            tc.tile_pool(name="ktc", bufs=2) as ktc_p,
            tc.tile_pool(name="vst", bufs=3) as vst_p,
            tc.tile_pool(name="qg", bufs=2) as qg_p,
            tc.tile_pool(name="rec", bufs=1) as rec_p,
            tc.tile_pool(name="res", bufs=2) as res_p,
            tc.tile_pool(name="ost", bufs=2) as ost_p,
            tc.tile_pool(name="spers", bufs=1, space="PSUM") as spers,
        ):
            sums23 = [
                spers.tile([P, TQ], F32, tag=f"sums{g}", name=f"sums{g}")
                for g in (2, 3)
            ]
            rec01 = rec_p.tile([P, 2, TQ], F32, tag="rec0")
            rec23 = rec_p.tile([P, 2, TQ], F32, tag="rec1")

            # ---- groups 0/1: fully AllGather-1 dependent ----
            with (
                tc.tile_pool(name="pt01", bufs=1) as pt01_p,
                tc.tile_pool(name="oa01", bufs=1) as oa01_p,
            ):
                pT01 = pt01_p.tile([P, 2 * NDIAG, 2, TQ], CDT, tag="pt")
                oaT01 = oa01_p.tile([P, DT, 2, TQ], CDT, tag="oa")
                with (
                    tc.tile_pool(name="psc", bufs=2, space="PSUM") as psc,
                    tc.tile_pool(name="psm", bufs=1, space="PSUM") as psm,
                ):
                    for g in (0, 1):
                        qg = load_qg(qg_p, g)
                        sums = psm.tile([P, TQ], F32, tag="sums",
                                        name=f"sums{g}")
                        m2_part(ktc_p, psc, g, qg, pT01, sums,
                                0, NDIAG * (g + 1))
                        nc.vector.reciprocal(out=rec01[:, g, :], in_=sums)

                def sink01(a, poa):
                    nc.vector.tensor_mul(
                        out=oaT01[:, a, :, :], in0=poa, in1=rec01
                    )

                with tc.tile_pool(name="poa", bufs=1, space="PSUM") as poa_p:
                    m3_sessions(vst_p, poa_p, 0, pT01, 0, 2 * NDIAG, sink01)
                with tc.tile_pool(name="pfin", bufs=2, space="PSUM") as pfin:
                    m4_group(res_p, ost_p, pfin, 0, oaT01)
                    m4_group(res_p, ost_p, pfin, 1, oaT01)

            # ---- groups 2/3: split across the AllGather-2 window ----
            with (
                tc.tile_pool(name="pt23", bufs=1) as pt23_p,
                tc.tile_pool(name="oa23", bufs=1) as oa23_p,
                tc.tile_pool(name="oal", bufs=1) as oal_p,
                tc.tile_pool(name="cmb", bufs=2) as cmb_p,
            ):
                pT23 = pt23_p.tile([P, 4 * NDIAG, 2, TQ], CDT, tag="pt")
                oaT23 = oa23_p.tile([P, DT, 2, TQ], CDT, tag="oa")
                oal23 = oal_p.tile([P, DT, 2, TQ], CDT, tag="oal")
                qg2 = load_qg(qg_p, 2)
                qg3 = load_qg(qg_p, 3)
                # window: chunk-1 scores for g2/g3
                with tc.tile_pool(name="psc2", bufs=2, space="PSUM") as psc2:
                    m2_part(ktc_p, psc2, 2, qg2, pT23, sums23[0], 0, 2 * NDIAG)
                    m2_part(ktc_p, psc2, 3, qg3, pT23, sums23[1], 0, 2 * NDIAG)

                # window: P@V partial over chunk-1 keys -> bf16
                def sink_lo(a, poa):
                    nc.vector.tensor_copy(oal23[:, a, :, :], poa)

                with tc.tile_pool(name="poa2", bufs=1, space="PSUM") as poa2_p:
                    m3_sessions(vst_p, poa2_p, 1, pT23, 0, 2 * NDIAG, sink_lo)

                # tail: AllGather-2 dependent
                with tc.tile_pool(name="psc3", bufs=2, space="PSUM") as psc3:
                    m2_part(ktc_p, psc3, 2, qg2, pT23, sums23[0],
                            2 * NDIAG, 3 * NDIAG)
                    nc.vector.reciprocal(out=rec23[:, 0, :], in_=sums23[0])
                    m2_part(ktc_p, psc3, 3, qg3, pT23, sums23[1],
                            2 * NDIAG, 4 * NDIAG)
                    nc.vector.reciprocal(out=rec23[:, 1, :], in_=sums23[1])

                def sink_hi(a, poa):
                    t = cmb_p.tile([P, 2, TQ], F32, tag="cmb")
                    nc.vector.tensor_add(out=t, in0=poa, in1=oal23[:, a, :, :])
                    nc.vector.tensor_mul(
                        out=oaT23[:, a, :, :], in0=t, in1=rec23
                    )

                with tc.tile_pool(name="poa3", bufs=1, space="PSUM") as poa3_p:
                    m3_sessions(vst_p, poa3_p, 1, pT23,
                                2 * NDIAG, 4 * NDIAG, sink_hi)
                with tc.tile_pool(name="pfin2", bufs=2, space="PSUM") as pfin2:
                    m4_group(res_p, ost_p, pfin2, 2, oaT23)
                    m4_group(res_p, ost_p, pfin2, 3, oaT23)

    if compile:
        nc.compile()
    return nc


def _make_masks(j):
    """Shared additive causal mask: m[r, s] = 0 iff r <= GQ*s + j - 896.
    Diagonal tile u uses the column slice [32*(NDIAG-1-u), +TQ)."""
    r = np.arange(P)[:, None]
    s = np.arange(MW)[None, :]
    return np.where(
        r <= GQ * s + j - GQ * 32 * (NDIAG - 1), 0.0, NEG
    ).astype(np.float32)


def _core_inputs(x, wq_h, wk_h, wv_h, wo_h, c):
    b, j = divmod(c, GQ)
    return {
        "xkv": np.concatenate(
            [
                x[b, CH * j : CH * (j + 1), :],
                x[b, S // 2 + CH * j : S // 2 + CH * (j + 1), :],
            ]
        ).astype(CDT_NP),
        "xq": np.ascontiguousarray(x[b, j::GQ, :]).astype(CDT_NP),
        "wq": wq_h,
        "wk": wk_h,
        "wv": wv_h,
        "wo": wo_h,
        "masks": _make_masks(j),
        "ident": np.eye(P, dtype=CDT_NP),
    }


_NC_CACHE = None
_last_in_maps = None


def kernel(x, qkv, o_proj):
    global _NC_CACHE
    if _NC_CACHE is None:
        _NC_CACHE = build_nc()
    nc = _NC_CACHE

    x = np.ascontiguousarray(np.asarray(x, dtype=np.float32))
    qkv = np.asarray(qkv, dtype=np.float32)
    o_proj = np.asarray(o_proj, dtype=np.float32)
    wq_h = np.ascontiguousarray(qkv[:, :D]).astype(CDT_NP)
    wk_h = np.ascontiguousarray(qkv[:, D : 2 * D]).astype(CDT_NP)
    wv_h = np.ascontiguousarray(qkv[:, 2 * D :]).astype(CDT_NP)
    wo_h = o_proj.astype(CDT_NP)

    in_maps = [
        _core_inputs(x, wq_h, wk_h, wv_h, wo_h, c) for c in range(NCORES)
    ]

    global _last_in_maps
    _last_in_maps = in_maps
    res = run_bass_kernel_spmd(nc, in_maps, list(range(NCORES)))

    outp = np.empty((B, S, H), dtype=np.float32)
    for c in range(NCORES):
        b, j = divmod(c, GQ)
        outp[b, j::GQ, :] = res.results[c]["out"]
    return outp


# revision 26
# speedup vs baseline: 1.1193x; 1.1193x over previous
"""Trainium2 Bass kernel for a single-head causal attention block.

Computes, per batch b:
    xn    = LayerNorm(x[b])           (non-affine, eps=1e-6)
    q,k,v = xn @ Wq, xn @ Wk, xn @ Wv
    s     = causal_mask(q @ k.T / sqrt(D))
    out   = softmax(s) @ v @ Wo + x[b]

Sharding (8 cores, SPMD single program):
  core c -> batch b = c//4, query stripe j = c%4 (queries {4k+j}).
  K/V ownership is chunked so the AllGather pipelines in two halves:
  core j computes K^T and V for tokens [512j, 512(j+1)) (chunk 1) and
  [2048+512j, 2048+512(j+1)) (chunk 2).  AllGather #1 delivers keys
  [0,2048) while chunk-2 K/V is still being computed; AllGather #2
  delivers keys [2048,4096) while attention on the first half runs.
  K is gathered already transposed (computed as K^T = Wk-tile.T @ xnT,
  same orientation as Q^T) so the attention phase needs no transposes.

Attention uses TQ=256 query groups (group g covers original positions
[1024g, 1024(g+1)) via the stride-4 query assignment): groups 0/1
depend only on AllGather #1; groups 2/3 accumulate P@V partial sums
over the first 16 key tiles inside the AllGather #2 window (saved to
bf16) and combine with the upper half afterwards.  Score tiles in the
causal-diagonal region with >=50% masked queries run at half moving
width.  Softmax denominators come from an all-ones stationary matmul
over pT accumulated in PSUM (lagged one tile behind exp so the PE
never waits on the Act engine); exp() is applied without max
subtraction (scores are O(10); fp32 exp cannot overflow), matching
softmax exactly after normalization.

The causal mask is shift-invariant across diagonal tiles:
mask[u][r, q] = 0 iff r <= 4q + j - 128u, so a single [128, 480] tile
m[r, s] = 0 iff r <= 4s + j - 896 serves every tile u via the column
slice [32*(7-u), 32*(7-u)+256).
"""

import numpy as np
import ml_dtypes

import concourse.bacc as bacc
import concourse.tile as tile
from concourse import mybir
from concourse.bass_utils import run_bass_kernel_spmd

# Problem shape (hardcoded per harness contract)
B, S, H, D = 2, 4096, 2048, 2048
NCORES = 8
P = 128            # partitions
GQ = NCORES // B   # cores per batch = query stride
SQ = S // GQ       # queries per core
TQ = 256           # query group width
NGRP = SQ // TQ    # query groups per core (4)
HT = H // P        # h tiles (16)
DT = D // P        # d tiles (16)
NDIAG = TQ * GQ // P   # diagonal (mask) key tiles per query group (8)
MW = TQ + 32 * (NDIAG - 1)  # shared mask tile width (480)
CH = 512           # tokens per core per chunk
CHEL = CH * D      # elements per core-chunk contribution (per tensor)
PAD = 128          # dummy tail elements chaining AllGather #2 after #1

F32 = mybir.dt.float32
BF16 = mybir.dt.bfloat16
CDT = BF16
CDT_NP = ml_dtypes.bfloat16

EPS = 1e-6
NEG = -1e30


def build_nc(compile=True):
    nc = bacc.Bacc(num_devices=NCORES)

    # I/O.  xkv rows = [chunk1 tokens ; chunk2 tokens] for this core.
    xkv = nc.dram_tensor("xkv", [2 * CH, H], CDT, kind="ExternalInput")
    xq = nc.dram_tensor("xq", [SQ, H], CDT, kind="ExternalInput")
    wq = nc.dram_tensor("wq", [H, D], CDT, kind="ExternalInput")
    wk = nc.dram_tensor("wk", [H, D], CDT, kind="ExternalInput")
    wv = nc.dram_tensor("wv", [H, D], CDT, kind="ExternalInput")
    wo = nc.dram_tensor("wo", [D, H], CDT, kind="ExternalInput")
    masks = nc.dram_tensor("masks", [P, MW], F32, kind="ExternalInput")
    ident = nc.dram_tensor("ident", [P, P], CDT, kind="ExternalInput")
    out = nc.dram_tensor("out", [SQ, H], F32, kind="ExternalOutput")

    # DRAM scratch
    xn_d = nc.dram_tensor("xn_d", [2 * CH, H], CDT)
    xqn_d = nc.dram_tensor("xqn_d", [SQ, H], CDT)
    qt_d = nc.dram_tensor("qt_d", [D, SQ], CDT)
    # Per-chunk gather buffers: [0] = K^T as [DT,128,CH], [1] = V as [CH,D].
    # Each sub-tensor carries a PAD-element tail; a dummy copy from agout0
    # into agin1's pad chains AllGather #2's inputs to AllGather #1's
    # completion, so the scheduling pass orders every DMA-ring slot the
    # same way the runtime serializes the collectives.
    agin = [nc.dram_tensor(f"agin{c}", [2, CHEL + PAD], CDT) for c in range(2)]
    agout = [
        nc.dram_tensor(f"agout{c}", [GQ, 2, CHEL + PAD], CDT) for c in range(2)
    ]
    CC_GROUPS = [list(range(g * GQ, (g + 1) * GQ)) for g in range(NCORES // GQ)]

    RSCALE = float(1.0 / np.sqrt(D))

    with (
        tile.TileContext(nc, pool_alloc_mode="queue") as tc,
        tc.tile_pool(name="consts", bufs=1) as consts,
        tc.tile_pool(name="wvo", bufs=1) as wvo_p,    # Wv then Wo slots
    ):
        ones = consts.tile([P, P], CDT)
        nc.vector.memset(ones, 1.0)
        eps_tile = consts.tile([P, 1], F32)
        nc.vector.memset(eps_tile, EPS)
        masks_sb = consts.tile([P, MW], F32)
        ident_sb = consts.tile([P, P], CDT)
        zpad = consts.tile([P, 1], CDT)
        nc.vector.memset(zpad, 0.0)

        def load_w(pool, w_dram, prefix, queues):
            tiles = []
            for a in range(HT):
                t = pool.tile([P, w_dram.shape[1]], CDT, tag=f"{prefix}{a}")
                q = queues[a % len(queues)]
                q.dma_start(out=t, in_=w_dram[a * P : (a + 1) * P, :])
                tiles.append(t)
            return tiles

        def load_xt(pool, src_d, row0):
            """[H, 512] block of x_norm^T via DMA crossbar transpose."""
            tiles = []
            for a in range(HT):
                t = pool.tile([P, CH], CDT, tag=f"xt{a}")
                nc.scalar.dma_start_transpose(
                    t, src_d[row0 : row0 + CH, a * P : (a + 1) * P]
                )
                tiles.append(t)
            return tiles

        # ======== Phase 1: LayerNorm, projections, gathers ========
        with (
            tc.tile_pool(name="wkq", bufs=1) as wkq_p,   # Wk then Wq slots
            tc.tile_pool(name="xnT", bufs=3) as xnT_p,
            tc.tile_pool(name="pp1", bufs=6, space="PSUM") as pp1,
            tc.tile_pool(name="ppt", bufs=2, space="PSUM") as ppt,
            tc.tile_pool(name="xtmp", bufs=2) as xtmp_p,
        ):
            with (
                tc.tile_pool(name="xpool", bufs=2) as xpool,
                tc.tile_pool(name="xnpool", bufs=2) as xnpool,
                tc.tile_pool(name="stats", bufs=2) as stats_p,
                tc.tile_pool(name="small", bufs=4) as small_p,
                tc.tile_pool(name="stage1", bufs=5) as stage_p,
            ):
                def ln_rows(src, dst, t0, nt, ql=None, qs=None):
                    """LayerNorm token tiles [t0, t0+nt) of src -> dst.
                    All x loads issued before any store (HOL-blocking)."""
                    ql = ql or nc.sync
                    qs = qs or nc.sync
                    xts = []
                    for t in range(t0, t0 + nt):
                        x_t = xpool.tile([P, H], CDT, tag="x")
                        ql.dma_start(
                            out=x_t, in_=src[t * P : (t + 1) * P, :]
                        )
                        xts.append(x_t)
                    for i, t in enumerate(range(t0, t0 + nt)):
                        x_t = xts[i]
                        stats = stats_p.tile([P, H // 512, 6], F32, tag="st")
                        for k in range(H // 512):
                            nc.vector.bn_stats(
                                out=stats[:, k, :],
                                in_=x_t[:, k * 512 : (k + 1) * 512],
                            )
                        mv = small_p.tile([P, 2], F32, tag="mv")
                        nc.vector.bn_aggr(out=mv, in_=stats)
                        sq = small_p.tile([P, 1], F32, tag="sq")
                        nc.scalar.activation(
                            out=sq, in_=mv[:, 1:2],
                            func=mybir.ActivationFunctionType.Sqrt,
                            bias=eps_tile, scale=1.0,
                        )
                        rs = small_p.tile([P, 1], F32, tag="rs")
                        nc.vector.reciprocal(out=rs, in_=sq)
                        xn_t = xnpool.tile([P, H], CDT, tag="xn")
                        nc.vector.tensor_scalar(
                            out=xn_t, in0=x_t, scalar1=mv[:, 0:1], scalar2=rs,
                            op0=mybir.AluOpType.subtract,
                            op1=mybir.AluOpType.mult,
                        )
                        qs.dma_start(
                            out=dst[t * P : (t + 1) * P, :], in_=xn_t
                        )

                def proj_kv(xt, ch, cpq):
                    """K^T and V projections for one chunk into agin[ch]."""
                    ktv = agin[ch][0, :CHEL].rearrange(
                        "(a p k) -> a p k", p=P, k=CH
                    )
                    vv = agin[ch][1, :CHEL].rearrange("(t d) -> t d", d=D)
                    # K^T: [128d, 512tok] tiles
                    for a in range(DT):
                        ps = pp1.tile([P, CH], F32, tag="ps")
                        for h in range(HT):
                            nc.tensor.matmul(
                                ps, wk_sb[h][:, a * P : (a + 1) * P], xt[h],
                                start=(h == 0), stop=(h == HT - 1),
                            )
                        st = stage_p.tile([P, CH], CDT, tag="st")
                        cpq(st, ps)
                        nc.sync.dma_start(out=ktv[a, :, :], in_=st)
                    # V: [128tok, 512d] tiles
                    for tl in range(CH // P):
                        for dc in range(D // 512):
                            ps = pp1.tile([P, 512], F32, tag="ps")
                            for h in range(HT):
                                nc.tensor.matmul(
                                    ps,
                                    xt[h][:, tl * P : (tl + 1) * P],
                                    wv_sb[h][:, dc * 512 : (dc + 1) * 512],
                                    start=(h == 0), stop=(h == HT - 1),
                                )
                            st = stage_p.tile([P, 512], CDT, tag="st")
                            cpq(st, ps)
                            nc.sync.dma_start(
                                out=vv[tl * P : (tl + 1) * P,
                                       dc * 512 : (dc + 1) * 512],
                                in_=st,
                            )

                def act_copy(st, ps):
                    nc.scalar.activation(
                        out=st, in_=ps,
                        func=mybir.ActivationFunctionType.Copy,
                        bias=0.0, scale=1.0,
                    )

                def proj_q(xt, qb):
                    """Q^T for query block qb (scaled by 1/sqrt(D))."""
                    for a in range(DT):
                        ps = pp1.tile([P, CH], F32, tag="ps")
                        for h in range(HT):
                            nc.tensor.matmul(
                                ps, wq_sb[h][:, a * P : (a + 1) * P], xt[h],
                                start=(h == 0), stop=(h == HT - 1),
                            )
                        st = stage_p.tile([P, CH], CDT, tag="st")
                        nc.scalar.activation(
                            out=st, in_=ps,
                            func=mybir.ActivationFunctionType.Copy,
                            bias=0.0, scale=RSCALE,
                        )
                        nc.sync.dma_start(
                            out=qt_d[a * P : (a + 1) * P,
                                     qb * CH : (qb + 1) * CH],
                            in_=st,
                        )

                def gather(ch):
                    nc.gpsimd.collective_compute(
                        "AllGather", mybir.AluOpType.bypass,
                        replica_groups=CC_GROUPS,
                        ins=[agin[ch][:, :]], outs=[agout[ch][:, :, :]],
                    )

                def tr_q(qb):
                    """xqn^T for query block qb via PE transposes (DMA
                    crossbar transposes serialize with collectives in the
                    scheduler's virtual queue -- any transpose whose pass
                    time races an AllGather stalls the DMA rings at
                    runtime, so only the early chunk transposes use the
                    crossbar)."""
                    tiles = [
                        xnT_p.tile([P, CH], CDT, tag=f"xt{a}",
                                   name=f"xtq{qb}_{a}")
                        for a in range(HT)
                    ]
                    for t in range(CH // P):
                        for hh in range(2):
                            xm = xtmp_p.tile([P, H // 2], CDT, tag="xm")
                            nc.scalar.dma_start(
                                out=xm,
                                in_=xqn_d[qb * CH + t * P : qb * CH + (t + 1) * P,
                                          hh * (H // 2) : (hh + 1) * (H // 2)],
                            )
                            for a8 in range(HT // 2):
                                a = hh * (HT // 2) + a8
                                ps = ppt.tile([P, P], CDT, tag="pt")
                                nc.tensor.transpose(
                                    ps, xm[:, a8 * P : (a8 + 1) * P], ident_sb
                                )
                                nc.vector.tensor_copy(
                                    tiles[a][:, t * P : (t + 1) * P], ps
                                )
                    return tiles

                wk_sb = load_w(wkq_p, wk, "k", [nc.gpsimd])
                nc.sync.dma_start(out=ident_sb, in_=ident[:, :])
                # zero the unwritten gather-pad tails (the agin1 V-pad is
                # written by the chain copy instead)
                for c, sub in ((0, 0), (0, 1), (1, 0)):
                    nc.sync.dma_start(
                        out=agin[c][sub, CHEL : CHEL + PAD].rearrange(
                            "(p f) -> p f", p=P
                        ),
                        in_=zpad,
                    )
                ln_rows(xkv, xn_d, 0, 4)
                ln_rows(xkv, xn_d, 4, 4)
                nc.sync.dma_start(out=masks_sb, in_=masks[:, :])
                wv_sb = load_w(wvo_p, wv, "v", [nc.gpsimd])

                xt_c1 = load_xt(xnT_p, xn_d, 0)
                # chunk-1 staging copies ride Act (the DVE still runs the
                # chunk-2/query LayerNorms); chunk-2 copies ride the DVE
                # (Act is blocked behind Wq's slot-reuse wait by then).
                proj_kv(xt_c1, 0, act_copy)
                # Query LayerNorm here: its x loads ride Act behind the
                # chunk-1 staging copies, stores ride Pool ahead of the
                # collectives, and the DVE work follows the chunk LNs.
                ln_rows(xq, xqn_d, 0, 4, ql=nc.scalar, qs=nc.gpsimd)
                ln_rows(xq, xqn_d, 4, 4, ql=nc.scalar, qs=nc.gpsimd)
                xt_c2 = load_xt(xnT_p, xn_d, CH)
                gather(0)
                # Wq on Act: its slot-reuse wait on the last Wk reader
                # would head-of-line-block SP's agin stores.
                wq_sb = load_w(wkq_p, wq, "k", [nc.scalar])
                proj_kv(xt_c2, 1, nc.vector.tensor_copy)
                xt_q1 = tr_q(0)
                proj_q(xt_q1, 0)
                xt_q2 = tr_q(1)
                proj_q(xt_q2, 1)
                # Chain AllGather #2's input set to AllGather #1's output so
                # the scheduling pass sees the same serialization the
                # COLLECTIVE_CORES resource enforces at runtime.
                nc.gpsimd.dma_start(
                    out=agin[1][1, CHEL : CHEL + PAD].rearrange(
                        "(p f) -> p f", p=P
                    ),
                    in_=agout[0][0, 0, 0:PAD].rearrange("(p f) -> p f", p=P),
                )
                gather(1)

        wo_sb = load_w(wvo_p, wo, "v", [nc.sync])  # reuse Wv slots

        # ======== Phase 2: attention ========
        def kt_batch(ktc_p, kc):
            """The 16 kT d-tiles for key batch kc (keys [512kc,+512))."""
            c, r = divmod(kc, 4)
            kv = agout[c][r, 0, :CHEL].rearrange("(a p k) -> a p k", p=P, k=CH)
            q = nc.sync if kc % 2 == 0 else nc.scalar
            kts = []
            for a in range(DT):
                t = ktc_p.tile([P, CH], CDT, tag=f"kt{a}")
                q.dma_start(out=t, in_=kv[a, :, :])
                kts.append(t)
            return kts

        def vt_load(vst_p, tk, d0):
            """V tile [128tok, 512] for key tile tk, d cols [d0,d0+512)."""
            c, q = divmod(tk, 16)
            vv = agout[c][q // 4, 1, :CHEL].rearrange("(t d) -> t d", d=D)
            row0 = (q % 4) * P
            t = vst_p.tile([P, 512], CDT, tag="vt")
            nc.scalar.dma_start(out=t, in_=vv[row0 : row0 + P, d0 : d0 + 512])
            return t

        def m2_part(ktc_p, psc, g, qg, pT, sums, tk0, tk1):
            """Score pass for group g over key tiles [tk0, tk1)."""
            TK = NDIAG * (g + 1)
            gh = g % 2
            lag = []

            def flush_lag():
                for s_tk, s_qoff, s_nw in lag:
                    nc.tensor.matmul(
                        sums[:, s_qoff : s_qoff + s_nw], ones,
                        pT[:, s_tk, gh, s_qoff : s_qoff + s_nw],
                        start=(s_tk == 0), stop=(s_tk == TK - 1),
                        skip_group_check=True,
                    )
                lag.clear()

            for kc in range(tk0 // 4, tk1 // 4):
                kts = kt_batch(ktc_p, kc)
                for t4 in range(4):
                    tk = kc * 4 + t4
                    u = tk - (TK - NDIAG)
                    qoff = P if (u >= 4) else 0
                    ps = psc.tile([P, TQ], F32, tag="ps")
                    for a in range(DT):
                        nc.tensor.matmul(
                            ps[:, qoff:],
                            kts[a][:, t4 * P : (t4 + 1) * P],
                            qg[:, a, qoff:],
                            start=(a == 0), stop=(a == DT - 1),
                        )
                    if u >= 0:
                        s0 = 32 * (NDIAG - 1 - u)
                        nc.vector.tensor_add(
                            out=ps[:, qoff:], in0=ps[:, qoff:],
                            in1=masks_sb[:, s0 + qoff : s0 + TQ],
                        )
                        if qoff:
                            nc.vector.memset(pT[:, tk, gh, 0:qoff], 0.0)
                    nc.scalar.activation(
                        out=pT[:, tk, gh, qoff:], in_=ps[:, qoff:],
                        func=mybir.ActivationFunctionType.Exp,
                    )
                    flush_lag()
                    lag.append((tk, qoff, TQ - qoff))
            flush_lag()

        def m3_sessions(vst_p, poa_p, pr, pT, tk0, tk1, sink):
            """P@V quarter-sessions for group pair pr over key tiles
            [tk0,tk1).  Tiles >= t_lo only feed the odd group.
            sink(a, psum_tile) consumes each finished d-tile."""
            t_lo = NDIAG * (2 * pr + 1)
            t_dg = NDIAG * (2 * pr + 1)  # odd group's diag base
            for qd in range(4):
                poas = [
                    poa_p.tile([P, 2, TQ], F32, tag=f"poa{d4}",
                               name=f"poa{pr}_{qd}_{d4}_{tk0}")
                    for d4 in range(4)
                ]
                for tk in range(tk0, tk1):
                    vt = vt_load(vst_p, tk, qd * 512)
                    for d4 in range(4):
                        if tk < t_lo:
                            o, r = poas[d4], pT[:, tk, :, :]
                        else:
                            qoff = P if (tk - t_dg >= 4) else 0
                            o = poas[d4][:, 1, qoff:]
                            r = pT[:, tk, 1, qoff:]
                        nc.tensor.matmul(
                            o, vt[:, d4 * P : (d4 + 1) * P], r,
                            start=(tk == tk0), stop=(tk == tk1 - 1),
                            skip_group_check=True,
                        )
                for d4 in range(4):
                    sink(qd * 4 + d4, poas[d4])

        def m4_group(res_p, ost_p, pfin, g, oaT):
            gh = g % 2
            for t2 in range(TQ // P):
                row0 = g * TQ + t2 * P
                for hc in range(H // 512):
                    ps = pfin.tile([P, 512], F32, tag="ps")
                    for d in range(DT):
                        nc.tensor.matmul(
                            ps,
                            oaT[:, d, gh, t2 * P : (t2 + 1) * P],
                            wo_sb[d][:, hc * 512 : (hc + 1) * 512],
                            start=(d == 0), stop=(d == DT - 1),
                        )
                    res = res_p.tile([P, 512], CDT, tag="res")
                    nc.sync.dma_start(
                        out=res,
                        in_=xq[row0 : row0 + P, hc * 512 : (hc + 1) * 512],
                    )
                    ot = ost_p.tile([P, 512], F32, tag="ot")
                    nc.vector.tensor_add(out=ot, in0=ps, in1=res)
                    nc.sync.dma_start(
                        out=out[row0 : row0 + P, hc * 512 : (hc + 1) * 512],
                        in_=ot,
                    )

        def load_qg(qg_p, g):
            t = qg_p.tile([P, DT, TQ], CDT, tag="qg", name=f"qg{g}")
            nc.sync.dma_start(
                out=t,
                in_=qt_d[:, g * TQ : (g + 1) * TQ].rearrange(
                    "(a p) t -> p a t", p=P
                ),
            )
            return t

        with (
            tc.tile_pool(name="ktc", bufs=2) as ktc_p,
            tc.tile_pool(name="vst", bufs=4) as vst_p,
            tc.tile_pool(name="qg", bufs=2) as qg_p,
            tc.tile_pool(name="rec", bufs=1) as rec_p,
            tc.tile_pool(name="res", bufs=2) as res_p,
            tc.tile_pool(name="ost", bufs=2) as ost_p,
            tc.tile_pool(name="spers", bufs=1, space="PSUM") as spers,
        ):
            sums23 = [
                spers.tile([P, TQ], F32, tag=f"sums{g}", name=f"sums{g}")
                for g in (2, 3)
            ]
            rec01 = rec_p.tile([P, 2, TQ], F32, tag="rec0")
            rec23 = rec_p.tile([P, 2, TQ], F32, tag="rec1")

            # ---- groups 0/1: fully AllGather-1 dependent ----
            with (
                tc.tile_pool(name="pt01", bufs=1) as pt01_p,
                tc.tile_pool(name="oa01", bufs=1) as oa01_p,
            ):
                pT01 = pt01_p.tile([P, 2 * NDIAG, 2, TQ], CDT, tag="pt")
                oaT01 = oa01_p.tile([P, DT, 2, TQ], CDT, tag="oa")
                with (
                    tc.tile_pool(name="psc", bufs=3, space="PSUM") as psc,
                    tc.tile_pool(name="psm", bufs=1, space="PSUM") as psm,
                ):
                    for g in (0, 1):
                        qg = load_qg(qg_p, g)
                        sums = psm.tile([P, TQ], F32, tag="sums",
                                        name=f"sums{g}")
                        m2_part(ktc_p, psc, g, qg, pT01, sums,
                                0, NDIAG * (g + 1))
                        nc.vector.reciprocal(out=rec01[:, g, :], in_=sums)

                def sink01(a, poa):
                    eng = nc.vector if a % 2 == 0 else nc.gpsimd
                    eng.tensor_mul(
                        out=oaT01[:, a, :, :], in0=poa, in1=rec01
                    )

                with tc.tile_pool(name="poa", bufs=1, space="PSUM") as poa_p:
                    m3_sessions(vst_p, poa_p, 0, pT01, 0, 2 * NDIAG, sink01)
                with tc.tile_pool(name="pfin", bufs=2, space="PSUM") as pfin:
                    m4_group(res_p, ost_p, pfin, 0, oaT01)
                    m4_group(res_p, ost_p, pfin, 1, oaT01)

            # ---- groups 2/3: split across the AllGather-2 window ----
            with (
                tc.tile_pool(name="pt23", bufs=1) as pt23_p,
                tc.tile_pool(name="oa23", bufs=1) as oa23_p,
                tc.tile_pool(name="oal", bufs=1) as oal_p,
                tc.tile_pool(name="cmb", bufs=2) as cmb_p,
            ):
                pT23 = pt23_p.tile([P, 4 * NDIAG, 2, TQ], CDT, tag="pt")
                oaT23 = oa23_p.tile([P, DT, 2, TQ], CDT, tag="oa")
                oal23 = oal_p.tile([P, DT, 2, TQ], CDT, tag="oal")
                qg2 = load_qg(qg_p, 2)
                qg3 = load_qg(qg_p, 3)
                # window: chunk-1 scores for g2/g3
                with tc.tile_pool(name="psc2", bufs=3, space="PSUM") as psc2:
                    m2_part(ktc_p, psc2, 2, qg2, pT23, sums23[0], 0, 2 * NDIAG)
                    m2_part(ktc_p, psc2, 3, qg3, pT23, sums23[1], 0, 2 * NDIAG)

                # window: P@V partial over chunk-1 keys -> bf16
                def sink_lo(a, poa):
                    eng = nc.vector if a % 2 == 0 else nc.gpsimd
                    eng.tensor_copy(oal23[:, a, :, :], poa)

                with tc.tile_pool(name="poa2", bufs=1, space="PSUM") as poa2_p:
                    m3_sessions(vst_p, poa2_p, 1, pT23, 0, 2 * NDIAG, sink_lo)

                # tail: AllGather-2 dependent.  Pin the tail's scheduling-
                # pass time past the whole window so no tail DMA can grab a
                # ring slot ahead of a window DMA (the runtime leak of the
                # pin only waits on window work that precedes AllGather #2
                # anyway).
                tc.tile_set_cur_wait(0.75)
                with tc.tile_pool(name="psc3", bufs=3, space="PSUM") as psc3:
                    m2_part(ktc_p, psc3, 2, qg2, pT23, sums23[0],
                            2 * NDIAG, 3 * NDIAG)
                    nc.vector.reciprocal(out=rec23[:, 0, :], in_=sums23[0])
                    m2_part(ktc_p, psc3, 3, qg3, pT23, sums23[1],
                            2 * NDIAG, 4 * NDIAG)
                    nc.vector.reciprocal(out=rec23[:, 1, :], in_=sums23[1])

                def sink_hi(a, poa):
                    eng = nc.vector if a % 2 == 0 else nc.gpsimd
                    t = cmb_p.tile([P, 2, TQ], F32, tag="cmb")
                    eng.tensor_add(out=t, in0=poa, in1=oal23[:, a, :, :])
                    eng.tensor_mul(
                        out=oaT23[:, a, :, :], in0=t, in1=rec23
                    )

                with tc.tile_pool(name="poa3", bufs=1, space="PSUM") as poa3_p:
                    m3_sessions(vst_p, poa3_p, 1, pT23,
                                2 * NDIAG, 4 * NDIAG, sink_hi)
                with tc.tile_pool(name="pfin2", bufs=2, space="PSUM") as pfin2:
                    m4_group(res_p, ost_p, pfin2, 2, oaT23)
                    m4_group(res_p, ost_p, pfin2, 3, oaT23)

    if compile:
        nc.compile()
    return nc


def _make_masks(j):
    """Shared additive causal mask: m[r, s] = 0 iff r <= GQ*s + j - 896.
    Diagonal tile u uses the column slice [32*(NDIAG-1-u), +TQ)."""
    r = np.arange(P)[:, None]
    s = np.arange(MW)[None, :]
    return np.where(
        r <= GQ * s + j - GQ * 32 * (NDIAG - 1), 0.0, NEG
    ).astype(np.float32)


def _core_inputs(x, wq_h, wk_h, wv_h, wo_h, c):
    b, j = divmod(c, GQ)
    return {
        "xkv": np.concatenate(
            [
                x[b, CH * j : CH * (j + 1), :],
                x[b, S // 2 + CH * j : S // 2 + CH * (j + 1), :],
            ]
        ).astype(CDT_NP),
        "xq": np.ascontiguousarray(x[b, j::GQ, :]).astype(CDT_NP),
        "wq": wq_h,
        "wk": wk_h,
        "wv": wv_h,
        "wo": wo_h,
        "masks": _make_masks(j),
        "ident": np.eye(P, dtype=CDT_NP),
    }


_NC_CACHE = None
_last_in_maps = None


def kernel(x, qkv, o_proj):
    global _NC_CACHE
    if _NC_CACHE is None:
        _NC_CACHE = build_nc()
    nc = _NC_CACHE

    x = np.ascontiguousarray(np.asarray(x, dtype=np.float32))
    qkv = np.asarray(qkv, dtype=np.float32)
    o_proj = np.asarray(o_proj, dtype=np.float32)
    wq_h = np.ascontiguousarray(qkv[:, :D]).astype(CDT_NP)
    wk_h = np.ascontiguousarray(qkv[:, D : 2 * D]).astype(CDT_NP)
    wv_h = np.ascontiguousarray(qkv[:, 2 * D :]).astype(CDT_NP)
    wo_h = o_proj.astype(CDT_NP)

    in_maps = [
        _core_inputs(x, wq_h, wk_h, wv_h, wo_h, c) for c in range(NCORES)
    ]

    global _last_in_maps
    _last_in_maps = in_maps
    res = run_bass_kernel_spmd(nc, in_maps, list(range(NCORES)))

    outp = np.empty((B, S, H), dtype=np.float32)
    for c in range(NCORES):
        b, j = divmod(c, GQ)
        outp[b, j::GQ, :] = res.results[c]["out"]
    return outp


# revision 28
# speedup vs baseline: 1.1327x; 1.0120x over previous
"""Trainium2 Bass kernel for a single-head causal attention block.

Computes, per batch b:
    xn    = LayerNorm(x[b])           (non-affine, eps=1e-6)
    q,k,v = xn @ Wq, xn @ Wk, xn @ Wv
    s     = causal_mask(q @ k.T / sqrt(D))
    out   = softmax(s) @ v @ Wo + x[b]

Sharding (8 cores, SPMD single program):
  core c -> batch b = c//4, query stripe j = c%4 (queries {4k+j}).
  K/V ownership is chunked so the AllGather pipelines in two halves:
  core j computes K^T and V for tokens [512j, 512(j+1)) (chunk 1) and
  [2048+512j, 2048+512(j+1)) (chunk 2).  AllGather #1 delivers keys
  [0,2048) while chunk-2 K/V is still being computed; AllGather #2
  delivers keys [2048,4096) while attention on the first half runs.
  K is gathered already transposed (computed as K^T = Wk-tile.T @ xnT,
  same orientation as Q^T) so the attention phase needs no transposes.

Attention uses TQ=256 query groups (group g covers original positions
[1024g, 1024(g+1)) via the stride-4 query assignment): groups 0/1
depend only on AllGather #1; groups 2/3 accumulate P@V partial sums
over the first 16 key tiles inside the AllGather #2 window (saved to
bf16) and combine with the upper half afterwards.  Score tiles in the
causal-diagonal region with >=50% masked queries run at half moving
width.  Softmax denominators come from an all-ones stationary matmul
over pT accumulated in PSUM (lagged one tile behind exp so the PE
never waits on the Act engine); exp() is applied without max
subtraction (scores are O(10); fp32 exp cannot overflow), matching
softmax exactly after normalization.

The causal mask is shift-invariant across diagonal tiles:
mask[u][r, q] = 0 iff r <= 4q + j - 128u, so a single [128, 480] tile
m[r, s] = 0 iff r <= 4s + j - 896 serves every tile u via the column
slice [32*(7-u), 32*(7-u)+256).
"""

import numpy as np
import ml_dtypes

import concourse.bacc as bacc
import concourse.tile as tile
from concourse import mybir
from concourse.bass_utils import run_bass_kernel_spmd

# Problem shape (hardcoded per harness contract)
B, S, H, D = 2, 4096, 2048, 2048
NCORES = 8
P = 128            # partitions
GQ = NCORES // B   # cores per batch = query stride
SQ = S // GQ       # queries per core
TQ = 256           # query group width
NGRP = SQ // TQ    # query groups per core (4)
HT = H // P        # h tiles (16)
DT = D // P        # d tiles (16)
NDIAG = TQ * GQ // P   # diagonal (mask) key tiles per query group (8)
MW = TQ + 32 * (NDIAG - 1)  # shared mask tile width (480)
CH = 512           # tokens per core per chunk
CHEL = CH * D      # elements per core-chunk contribution (per tensor)
PAD = 128          # dummy tail elements chaining AllGather #2 after #1

F32 = mybir.dt.float32
BF16 = mybir.dt.bfloat16
CDT = BF16
CDT_NP = ml_dtypes.bfloat16

EPS = 1e-6
NEG = -1e30


def build_nc(compile=True):
    nc = bacc.Bacc(num_devices=NCORES)

    # I/O.  xkv rows = [chunk1 tokens ; chunk2 tokens] for this core.
    xkv = nc.dram_tensor("xkv", [2 * CH, H], CDT, kind="ExternalInput")
    xq = nc.dram_tensor("xq", [SQ, H], CDT, kind="ExternalInput")
    wq = nc.dram_tensor("wq", [H, D], CDT, kind="ExternalInput")
    wk = nc.dram_tensor("wk", [H, D], CDT, kind="ExternalInput")
    wv = nc.dram_tensor("wv", [H, D], CDT, kind="ExternalInput")
    wo = nc.dram_tensor("wo", [D, H], CDT, kind="ExternalInput")
    masks = nc.dram_tensor("masks", [P, MW], F32, kind="ExternalInput")
    ident = nc.dram_tensor("ident", [P, P], CDT, kind="ExternalInput")
    out = nc.dram_tensor("out", [SQ, H], F32, kind="ExternalOutput")

    # DRAM scratch
    xn_d = nc.dram_tensor("xn_d", [2 * CH, H], CDT)
    xqn_d = nc.dram_tensor("xqn_d", [SQ, H], CDT)
    qt_d = nc.dram_tensor("qt_d", [D, SQ], CDT)
    # Per-chunk gather buffers: [0] = K^T as [DT,128,CH], [1] = V as [CH,D].
    # Each sub-tensor carries a PAD-element tail; a dummy copy from agout0
    # into agin1's pad chains AllGather #2's inputs to AllGather #1's
    # completion, so the scheduling pass orders every DMA-ring slot the
    # same way the runtime serializes the collectives.
    agin = [nc.dram_tensor(f"agin{c}", [2, CHEL + PAD], CDT) for c in range(2)]
    agout = [
        nc.dram_tensor(f"agout{c}", [GQ, 2, CHEL + PAD], CDT) for c in range(2)
    ]
    CC_GROUPS = [list(range(g * GQ, (g + 1) * GQ)) for g in range(NCORES // GQ)]

    RSCALE = float(1.0 / np.sqrt(D))

    with (
        tile.TileContext(nc, pool_alloc_mode="queue") as tc,
        tc.tile_pool(name="consts", bufs=1) as consts,
        tc.tile_pool(name="wvo", bufs=1) as wvo_p,    # Wv then Wo slots
    ):
        ones = consts.tile([P, P], CDT)
        nc.vector.memset(ones, 1.0)
        eps_tile = consts.tile([P, 1], F32)
        nc.vector.memset(eps_tile, EPS)
        masks_sb = consts.tile([P, MW], F32)
        ident_sb = consts.tile([P, P], CDT)
        zpad = consts.tile([P, 1], CDT)
        nc.vector.memset(zpad, 0.0)

        def load_w(pool, w_dram, prefix, queues):
            tiles = []
            for a in range(HT):
                t = pool.tile([P, w_dram.shape[1]], CDT, tag=f"{prefix}{a}")
                q = queues[a % len(queues)]
                q.dma_start(out=t, in_=w_dram[a * P : (a + 1) * P, :])
                tiles.append(t)
            return tiles

        def load_xt(pool, src_d, row0):
            """[H, 512] block of x_norm^T via DMA crossbar transpose."""
            tiles = []
            for a in range(HT):
                t = pool.tile([P, CH], CDT, tag=f"xt{a}")
                nc.scalar.dma_start_transpose(
                    t, src_d[row0 : row0 + CH, a * P : (a + 1) * P]
                )
                tiles.append(t)
            return tiles

        # ======== Phase 1: LayerNorm, projections, gathers ========
        with (
            tc.tile_pool(name="wkq", bufs=1) as wkq_p,   # Wk then Wq slots
            tc.tile_pool(name="xnT", bufs=3) as xnT_p,
            tc.tile_pool(name="pp1", bufs=6, space="PSUM") as pp1,
            tc.tile_pool(name="ppt", bufs=2, space="PSUM") as ppt,
            tc.tile_pool(name="xtmp", bufs=2) as xtmp_p,
        ):
            with (
                tc.tile_pool(name="xpool", bufs=2) as xpool,
                tc.tile_pool(name="xnpool", bufs=2) as xnpool,
                tc.tile_pool(name="stats", bufs=2) as stats_p,
                tc.tile_pool(name="small", bufs=4) as small_p,
                tc.tile_pool(name="stage1", bufs=8) as stage_p,
            ):
                def ln_rows(src, dst, t0, nt, ql=None, qs=None):
                    """LayerNorm token tiles [t0, t0+nt) of src -> dst.
                    All x loads issued before any store (HOL-blocking)."""
                    ql = ql or nc.sync
                    qs = qs or nc.sync
                    xts = []
                    for t in range(t0, t0 + nt):
                        x_t = xpool.tile([P, H], CDT, tag="x")
                        ql.dma_start(
                            out=x_t, in_=src[t * P : (t + 1) * P, :]
                        )
                        xts.append(x_t)
                    for i, t in enumerate(range(t0, t0 + nt)):
                        x_t = xts[i]
                        stats = stats_p.tile([P, H // 512, 6], F32, tag="st")
                        for k in range(H // 512):
                            nc.vector.bn_stats(
                                out=stats[:, k, :],
                                in_=x_t[:, k * 512 : (k + 1) * 512],
                            )
                        mv = small_p.tile([P, 2], F32, tag="mv")
                        nc.vector.bn_aggr(out=mv, in_=stats)
                        sq = small_p.tile([P, 1], F32, tag="sq")
                        nc.scalar.activation(
                            out=sq, in_=mv[:, 1:2],
                            func=mybir.ActivationFunctionType.Sqrt,
                            bias=eps_tile, scale=1.0,
                        )
                        rs = small_p.tile([P, 1], F32, tag="rs")
                        nc.vector.reciprocal(out=rs, in_=sq)
                        xn_t = xnpool.tile([P, H], CDT, tag="xn")
                        nc.vector.tensor_scalar(
                            out=xn_t, in0=x_t, scalar1=mv[:, 0:1], scalar2=rs,
                            op0=mybir.AluOpType.subtract,
                            op1=mybir.AluOpType.mult,
                        )
                        qs.dma_start(
                            out=dst[t * P : (t + 1) * P, :], in_=xn_t
                        )

                def proj_kv(xt, ch, cpq):
                    """K^T and V projections for one chunk into agin[ch]."""
                    ktv = agin[ch][0, :CHEL].rearrange(
                        "(a p k) -> a p k", p=P, k=CH
                    )
                    vv = agin[ch][1, :CHEL].rearrange("(t d) -> t d", d=D)
                    # K^T: [128d, 512tok] tiles
                    for a in range(DT):
                        ps = pp1.tile([P, CH], F32, tag="ps")
                        for h in range(HT):
                            nc.tensor.matmul(
                                ps, wk_sb[h][:, a * P : (a + 1) * P], xt[h],
                                start=(h == 0), stop=(h == HT - 1),
                            )
                        st = stage_p.tile([P, CH], CDT, tag="st")
                        cpq(st, ps)
                        nc.sync.dma_start(out=ktv[a, :, :], in_=st)
                    # V: [128tok, 512d] tiles
                    for tl in range(CH // P):
                        for dc in range(D // 512):
                            ps = pp1.tile([P, 512], F32, tag="ps")
                            for h in range(HT):
                                nc.tensor.matmul(
                                    ps,
                                    xt[h][:, tl * P : (tl + 1) * P],
                                    wv_sb[h][:, dc * 512 : (dc + 1) * 512],
                                    start=(h == 0), stop=(h == HT - 1),
                                )
                            st = stage_p.tile([P, 512], CDT, tag="st")
                            cpq(st, ps)
                            nc.sync.dma_start(
                                out=vv[tl * P : (tl + 1) * P,
                                       dc * 512 : (dc + 1) * 512],
                                in_=st,
                            )

                def act_copy(st, ps):
                    nc.scalar.activation(
                        out=st, in_=ps,
                        func=mybir.ActivationFunctionType.Copy,
                        bias=0.0, scale=1.0,
                    )

                def proj_q(xt, qb):
                    """Q^T for query block qb (scaled by 1/sqrt(D))."""
                    for a in range(DT):
                        ps = pp1.tile([P, CH], F32, tag="ps")
                        for h in range(HT):
                            nc.tensor.matmul(
                                ps, wq_sb[h][:, a * P : (a + 1) * P], xt[h],
                                start=(h == 0), stop=(h == HT - 1),
                            )
                        st = stage_p.tile([P, CH], CDT, tag="st")
                        nc.scalar.activation(
                            out=st, in_=ps,
                            func=mybir.ActivationFunctionType.Copy,
                            bias=0.0, scale=RSCALE,
                        )
                        nc.sync.dma_start(
                            out=qt_d[a * P : (a + 1) * P,
                                     qb * CH : (qb + 1) * CH],
                            in_=st,
                        )

                def gather(ch):
                    nc.gpsimd.collective_compute(
                        "AllGather", mybir.AluOpType.bypass,
                        replica_groups=CC_GROUPS,
                        ins=[agin[ch][:, :]], outs=[agout[ch][:, :, :]],
                    )

                def tr_q(qb):
                    """xqn^T for query block qb via PE transposes (DMA
                    crossbar transposes serialize with collectives in the
                    scheduler's virtual queue -- any transpose whose pass
                    time races an AllGather stalls the DMA rings at
                    runtime, so only the early chunk transposes use the
                    crossbar)."""
                    tiles = [
                        xnT_p.tile([P, CH], CDT, tag=f"xt{a}",
                                   name=f"xtq{qb}_{a}")
                        for a in range(HT)
                    ]
                    for t in range(CH // P):
                        for hh in range(2):
                            xm = xtmp_p.tile([P, H // 2], CDT, tag="xm")
                            nc.scalar.dma_start(
                                out=xm,
                                in_=xqn_d[qb * CH + t * P : qb * CH + (t + 1) * P,
                                          hh * (H // 2) : (hh + 1) * (H // 2)],
                            )
                            for a8 in range(HT // 2):
                                a = hh * (HT // 2) + a8
                                ps = ppt.tile([P, P], CDT, tag="pt")
                                nc.tensor.transpose(
                                    ps, xm[:, a8 * P : (a8 + 1) * P], ident_sb
                                )
                                nc.vector.tensor_copy(
                                    tiles[a][:, t * P : (t + 1) * P], ps
                                )
                    return tiles

                wk_sb = load_w(wkq_p, wk, "k", [nc.gpsimd])
                nc.sync.dma_start(out=ident_sb, in_=ident[:, :])
                # zero the unwritten gather-pad tails (the agin1 V-pad is
                # written by the chain copy instead)
                for c, sub in ((0, 0), (0, 1), (1, 0)):
                    nc.sync.dma_start(
                        out=agin[c][sub, CHEL : CHEL + PAD].rearrange(
                            "(p f) -> p f", p=P
                        ),
                        in_=zpad,
                    )
                ln_rows(xkv, xn_d, 0, 4)
                ln_rows(xkv, xn_d, 4, 4)
                nc.sync.dma_start(out=masks_sb, in_=masks[:, :])
                wv_sb = load_w(wvo_p, wv, "v", [nc.gpsimd])

                xt_c1 = load_xt(xnT_p, xn_d, 0)
                # chunk-1 staging copies ride Act (the DVE still runs the
                # chunk-2/query LayerNorms); chunk-2 copies ride the DVE
                # (Act is blocked behind Wq's slot-reuse wait by then).
                proj_kv(xt_c1, 0, act_copy)
                # Query LayerNorm here: its x loads ride Act behind the
                # chunk-1 staging copies, stores ride Pool ahead of the
                # collectives, and the DVE work follows the chunk LNs.
                ln_rows(xq, xqn_d, 0, 4, ql=nc.scalar, qs=nc.gpsimd)
                ln_rows(xq, xqn_d, 4, 4, ql=nc.scalar, qs=nc.gpsimd)
                xt_c2 = load_xt(xnT_p, xn_d, CH)
                gather(0)
                # Wq on Act: its slot-reuse wait on the last Wk reader
                # would head-of-line-block SP's agin stores.
                wq_sb = load_w(wkq_p, wq, "k", [nc.scalar])
                proj_kv(xt_c2, 1, nc.vector.tensor_copy)
                xt_q1 = tr_q(0)
                proj_q(xt_q1, 0)
                xt_q2 = tr_q(1)
                proj_q(xt_q2, 1)
                # Chain AllGather #2's input set to AllGather #1's output so
                # the scheduling pass sees the same serialization the
                # COLLECTIVE_CORES resource enforces at runtime.
                nc.gpsimd.dma_start(
                    out=agin[1][1, CHEL : CHEL + PAD].rearrange(
                        "(p f) -> p f", p=P
                    ),
                    in_=agout[0][0, 0, 0:PAD].rearrange("(p f) -> p f", p=P),
                )
                gather(1)

        wo_sb = load_w(wvo_p, wo, "v", [nc.sync])  # reuse Wv slots

        # ======== Phase 2: attention ========
        def kt_batch(ktc_p, kc):
            """The 16 kT d-tiles for key batch kc (keys [512kc,+512))."""
            c, r = divmod(kc, 4)
            kv = agout[c][r, 0, :CHEL].rearrange("(a p k) -> a p k", p=P, k=CH)
            q = nc.sync if kc % 2 == 0 else nc.scalar
            kts = []
            for a in range(DT):
                t = ktc_p.tile([P, CH], CDT, tag=f"kt{a}")
                q.dma_start(out=t, in_=kv[a, :, :])
                kts.append(t)
            return kts

        def vt_load(vst_p, tk, d0):
            """V tile [128tok, 512] for key tile tk, d cols [d0,d0+512)."""
            c, q = divmod(tk, 16)
            vv = agout[c][q // 4, 1, :CHEL].rearrange("(t d) -> t d", d=D)
            row0 = (q % 4) * P
            t = vst_p.tile([P, 512], CDT, tag="vt")
            nc.scalar.dma_start(out=t, in_=vv[row0 : row0 + P, d0 : d0 + 512])
            return t

        def m2_part(ktc_p, psc, g, qg, pT, sums, tk0, tk1):
            """Score pass for group g over key tiles [tk0, tk1)."""
            TK = NDIAG * (g + 1)
            gh = g % 2
            lag = []

            def flush_lag():
                for s_tk, s_qoff, s_nw in lag:
                    nc.tensor.matmul(
                        sums[:, s_qoff : s_qoff + s_nw], ones,
                        pT[:, s_tk, gh, s_qoff : s_qoff + s_nw],
                        start=(s_tk == 0), stop=(s_tk == TK - 1),
                        skip_group_check=True,
                    )
                lag.clear()

            for kc in range(tk0 // 4, tk1 // 4):
                kts = kt_batch(ktc_p, kc)
                for t4 in range(4):
                    tk = kc * 4 + t4
                    u = tk - (TK - NDIAG)
                    qoff = P if (u >= 4) else 0
                    ps = psc.tile([P, TQ], F32, tag="ps")
                    for a in range(DT):
                        nc.tensor.matmul(
                            ps[:, qoff:],
                            kts[a][:, t4 * P : (t4 + 1) * P],
                            qg[:, a, qoff:],
                            start=(a == 0), stop=(a == DT - 1),
                        )
                    if u >= 0:
                        s0 = 32 * (NDIAG - 1 - u)
                        nc.vector.tensor_add(
                            out=ps[:, qoff:], in0=ps[:, qoff:],
                            in1=masks_sb[:, s0 + qoff : s0 + TQ],
                        )
                        if qoff:
                            nc.vector.memset(pT[:, tk, gh, 0:qoff], 0.0)
                    nc.scalar.activation(
                        out=pT[:, tk, gh, qoff:], in_=ps[:, qoff:],
                        func=mybir.ActivationFunctionType.Exp,
                    )
                    flush_lag()
                    lag.append((tk, qoff, TQ - qoff))
            flush_lag()

        def m3_sessions(vst_p, poa_p, pr, pT, tk0, tk1, sink):
            """P@V quarter-sessions for group pair pr over key tiles
            [tk0,tk1).  Tiles >= t_lo only feed the odd group.
            sink(a, psum_tile) consumes each finished d-tile."""
            t_lo = NDIAG * (2 * pr + 1)
            t_dg = NDIAG * (2 * pr + 1)  # odd group's diag base
            for qd in range(4):
                poas = [
                    poa_p.tile([P, 2, TQ], F32, tag=f"poa{d4}",
                               name=f"poa{pr}_{qd}_{d4}_{tk0}")
                    for d4 in range(4)
                ]
                for tk in range(tk0, tk1):
                    vt = vt_load(vst_p, tk, qd * 512)
                    for d4 in range(4):
                        if tk < t_lo:
                            o, r = poas[d4], pT[:, tk, :, :]
                        else:
                            qoff = P if (tk - t_dg >= 4) else 0
                            o = poas[d4][:, 1, qoff:]
                            r = pT[:, tk, 1, qoff:]
                        nc.tensor.matmul(
                            o, vt[:, d4 * P : (d4 + 1) * P], r,
                            start=(tk == tk0), stop=(tk == tk1 - 1),
                            skip_group_check=True,
                        )
                for d4 in range(4):
                    sink(qd * 4 + d4, poas[d4])

        def m4_group(res_p, ost_p, pfin, g, oaT):
            gh = g % 2
            for t2 in range(TQ // P):
                row0 = g * TQ + t2 * P
                for hc in range(H // 512):
                    ps = pfin.tile([P, 512], F32, tag="ps")
                    for d in range(DT):
                        nc.tensor.matmul(
                            ps,
                            oaT[:, d, gh, t2 * P : (t2 + 1) * P],
                            wo_sb[d][:, hc * 512 : (hc + 1) * 512],
                            start=(d == 0), stop=(d == DT - 1),
                        )
                    res = res_p.tile([P, 512], CDT, tag="res")
                    nc.sync.dma_start(
                        out=res,
                        in_=xq[row0 : row0 + P, hc * 512 : (hc + 1) * 512],
                    )
                    ot = ost_p.tile([P, 512], F32, tag="ot")
                    nc.vector.tensor_add(out=ot, in0=ps, in1=res)
                    nc.sync.dma_start(
                        out=out[row0 : row0 + P, hc * 512 : (hc + 1) * 512],
                        in_=ot,
                    )

        def load_qg(qg_p, g):
            t = qg_p.tile([P, DT, TQ], CDT, tag="qg", name=f"qg{g}")
            nc.sync.dma_start(
                out=t,
                in_=qt_d[:, g * TQ : (g + 1) * TQ].rearrange(
                    "(a p) t -> p a t", p=P
                ),
            )
            return t

        with (
            tc.tile_pool(name="ktc", bufs=2) as ktc_p,
            tc.tile_pool(name="vst", bufs=4) as vst_p,
            tc.tile_pool(name="qg", bufs=2) as qg_p,
            tc.tile_pool(name="rec", bufs=1) as rec_p,
            tc.tile_pool(name="res", bufs=2) as res_p,
            tc.tile_pool(name="ost", bufs=2) as ost_p,
            tc.tile_pool(name="spers", bufs=1, space="PSUM") as spers,
        ):
            sums23 = [
                spers.tile([P, TQ], F32, tag=f"sums{g}", name=f"sums{g}")
                for g in (2, 3)
            ]
            rec01 = rec_p.tile([P, 2, TQ], F32, tag="rec0")
            rec23 = rec_p.tile([P, 2, TQ], F32, tag="rec1")

            # ---- groups 0/1: fully AllGather-1 dependent ----
            with (
                tc.tile_pool(name="pt01", bufs=1) as pt01_p,
                tc.tile_pool(name="oa01", bufs=1) as oa01_p,
            ):
                pT01 = pt01_p.tile([P, 2 * NDIAG, 2, TQ], CDT, tag="pt")
                oaT01 = oa01_p.tile([P, DT, 2, TQ], CDT, tag="oa")
                with (
                    tc.tile_pool(name="psc", bufs=3, space="PSUM") as psc,
                    tc.tile_pool(name="psm", bufs=1, space="PSUM") as psm,
                ):
                    for g in (0, 1):
                        qg = load_qg(qg_p, g)
                        sums = psm.tile([P, TQ], F32, tag="sums",
                                        name=f"sums{g}")
                        m2_part(ktc_p, psc, g, qg, pT01, sums,
                                0, NDIAG * (g + 1))
                        nc.vector.reciprocal(out=rec01[:, g, :], in_=sums)

                def sink01(a, poa):
                    nc.vector.tensor_mul(
                        out=oaT01[:, a, :, :], in0=poa, in1=rec01
                    )

                with tc.tile_pool(name="poa", bufs=1, space="PSUM") as poa_p:
                    m3_sessions(vst_p, poa_p, 0, pT01, 0, 2 * NDIAG, sink01)
                with tc.tile_pool(name="pfin", bufs=2, space="PSUM") as pfin:
                    m4_group(res_p, ost_p, pfin, 0, oaT01)
                    m4_group(res_p, ost_p, pfin, 1, oaT01)

            # ---- groups 2/3: split across the AllGather-2 window ----
            with (
                tc.tile_pool(name="pt23", bufs=1) as pt23_p,
                tc.tile_pool(name="oa23", bufs=1) as oa23_p,
                tc.tile_pool(name="oal", bufs=1) as oal_p,
                tc.tile_pool(name="cmb", bufs=2) as cmb_p,
            ):
                pT23 = pt23_p.tile([P, 4 * NDIAG, 2, TQ], CDT, tag="pt")
                oaT23 = oa23_p.tile([P, DT, 2, TQ], CDT, tag="oa")
                oal23 = oal_p.tile([P, DT, 2, TQ], CDT, tag="oal")
                qg2 = load_qg(qg_p, 2)
                qg3 = load_qg(qg_p, 3)
                # window: chunk-1 scores for g2/g3
                with tc.tile_pool(name="psc2", bufs=3, space="PSUM") as psc2:
                    m2_part(ktc_p, psc2, 2, qg2, pT23, sums23[0], 0, 2 * NDIAG)
                    m2_part(ktc_p, psc2, 3, qg3, pT23, sums23[1], 0, 2 * NDIAG)

                # window: P@V partial over chunk-1 keys -> bf16
                def sink_lo(a, poa):
                    nc.vector.tensor_copy(oal23[:, a, :, :], poa)

                with tc.tile_pool(name="poa2", bufs=1, space="PSUM") as poa2_p:
                    m3_sessions(vst_p, poa2_p, 1, pT23, 0, 2 * NDIAG, sink_lo)

                # tail: AllGather-2 dependent.  Pin the tail's scheduling-
                # pass time past the whole window so no tail DMA can grab a
                # ring slot ahead of a window DMA (the runtime leak of the
                # pin only waits on window work that precedes AllGather #2
                # anyway).
                tc.tile_set_cur_wait(0.75)
                with tc.tile_pool(name="psc3", bufs=3, space="PSUM") as psc3:
                    m2_part(ktc_p, psc3, 2, qg2, pT23, sums23[0],
                            2 * NDIAG, 3 * NDIAG)
                    nc.vector.reciprocal(out=rec23[:, 0, :], in_=sums23[0])
                    m2_part(ktc_p, psc3, 3, qg3, pT23, sums23[1],
                            2 * NDIAG, 4 * NDIAG)
                    nc.vector.reciprocal(out=rec23[:, 1, :], in_=sums23[1])

                def sink_hi(a, poa):
                    t = cmb_p.tile([P, 2, TQ], F32, tag="cmb")
                    nc.vector.tensor_add(out=t, in0=poa, in1=oal23[:, a, :, :])
                    nc.vector.tensor_mul(
                        out=oaT23[:, a, :, :], in0=t, in1=rec23
                    )

                with tc.tile_pool(name="poa3", bufs=1, space="PSUM") as poa3_p:
                    m3_sessions(vst_p, poa3_p, 1, pT23,
                                2 * NDIAG, 4 * NDIAG, sink_hi)
                with tc.tile_pool(name="pfin2", bufs=2, space="PSUM") as pfin2:
                    m4_group(res_p, ost_p, pfin2, 2, oaT23)
                    m4_group(res_p, ost_p, pfin2, 3, oaT23)

    if compile:
        nc.compile()
    return nc


def _make_masks(j):
    """Shared additive causal mask: m[r, s] = 0 iff r <= GQ*s + j - 896.
    Diagonal tile u uses the column slice [32*(NDIAG-1-u), +TQ)."""
    r = np.arange(P)[:, None]
    s = np.arange(MW)[None, :]
    return np.where(
        r <= GQ * s + j - GQ * 32 * (NDIAG - 1), 0.0, NEG
    ).astype(np.float32)


def _core_inputs(x, wq_h, wk_h, wv_h, wo_h, c):
    b, j = divmod(c, GQ)
    return {
        "xkv": np.concatenate(
            [
                x[b, CH * j : CH * (j + 1), :],
                x[b, S // 2 + CH * j : S // 2 + CH * (j + 1), :],
            ]
        ).astype(CDT_NP),
        "xq": np.ascontiguousarray(x[b, j::GQ, :]).astype(CDT_NP),
        "wq": wq_h,
        "wk": wk_h,
        "wv": wv_h,
        "wo": wo_h,
        "masks": _make_masks(j),
        "ident": np.eye(P, dtype=CDT_NP),
    }


_NC_CACHE = None
_last_in_maps = None


def kernel(x, qkv, o_proj):
    global _NC_CACHE
    if _NC_CACHE is None:
        _NC_CACHE = build_nc()
    nc = _NC_CACHE

    x = np.ascontiguousarray(np.asarray(x, dtype=np.float32))
    qkv = np.asarray(qkv, dtype=np.float32)
    o_proj = np.asarray(o_proj, dtype=np.float32)
    wq_h = np.ascontiguousarray(qkv[:, :D]).astype(CDT_NP)
    wk_h = np.ascontiguousarray(qkv[:, D : 2 * D]).astype(CDT_NP)
    wv_h = np.ascontiguousarray(qkv[:, 2 * D :]).astype(CDT_NP)
    wo_h = o_proj.astype(CDT_NP)

    in_maps = [
        _core_inputs(x, wq_h, wk_h, wv_h, wo_h, c) for c in range(NCORES)
    ]

    global _last_in_maps
    _last_in_maps = in_maps
    res = run_bass_kernel_spmd(nc, in_maps, list(range(NCORES)))

    outp = np.empty((B, S, H), dtype=np.float32)
    for c in range(NCORES):
        b, j = divmod(c, GQ)
        outp[b, j::GQ, :] = res.results[c]["out"]
    return outp
